# revision 27
# baseline (speedup 1.0000x reference)
"""V4: fp16-output MoE routing kernel.

The correctness gate is rel_err < 2e-2; plain fp16 arithmetic gives ~1.4e-3,
so no Dekker splits; fp16 output halves HBM write traffic. Engine budget per
core (125,952 tokens): ACT tanh ~116us gapless phase (bottleneck), DMA
~107us, DVE ~60us, PE ~55us. v4 focuses on the startup ramp (22.9us in v3)
and tail.

Data flow per 512-token pack:
- P[p, 32g+s] fp16 slot matrix, 9 slots used per 32-slot group, built by a
  SINGLE scalar_tensor_tensor per supertile: (btE == 1) * fx, where the host
  ships btE = bus_type - e and fx = [f0, f1, 1] triples per expert block so
  the DVE write runs are 9 contiguous elements (s = 3e+j).
- DVE stream-transpose (32x32 blocks): X32[32A+i, 32B+j] = P[32A+j, 32B+i]
- matmul(out, lhsT=X32, rhs=Wbig) with Wbig block-diagonal over A-blocks:
    out[32B+j, A*128+d] = z(token(p=32A+j, g=B), d)    (one PSUM bank/pack)
- ACT tanh over 4 packs (FD=2048, 4 PSUM banks double-buffered), fp16 SBUF
- output dumped linearly [128, npk*512]; the host input permutation is
  chosen so per-partition HBM lines are contiguous and the host-side unpack
  is a pure reshape: orig token q*(npk*4) + pt*4 + A sits at device slot
  (p = 32A + (q%32), pack pt, g = q//32).
"""

import sys
from contextlib import ExitStack

import numpy as np

sys.path.insert(0, "/opt/trn_rl_repo")

import concourse.bacc as bacc  # noqa: E402
import concourse.mybir as mybir  # noqa: E402
import concourse.tile as tile  # noqa: E402
from concourse.bass_utils import run_bass_kernel_spmd  # noqa: E402

FP = mybir.dt.float32
F16 = mybir.dt.float16
D = 128
PACK = 512
SUPERS = [16384] * 7 + [11264]
NPKS = [s // PACK for s in SUPERS]
N_CORES = 8
PER_CORE = sum(SUPERS)

_NC_CACHE = {}


def _body(ctx, tc, out, btc, fxc, wbig):
    nc = tc.nc
    eq = mybir.AluOpType.is_equal
    mult = mybir.AluOpType.mult

    const_pool = ctx.enter_context(tc.tile_pool(name="const", bufs=1))
    wbig_sb = const_pool.tile([128, 512], F16)
    nc.sync.dma_start(wbig_sb[:], wbig)
    # Persistent double-buffered slot matrices; slots 9..31 are zeroed once
    # and never written again (their Wbig rows are zero, but NaN garbage
    # would still poison the accumulation, so the memset is required).
    P_tiles = [const_pool.tile([128, 32 * 128], F16, name=f"Pbuf{i}")
               for i in range(2)]
    # Zeroed dummy operand lets PE warmup start right after the preamble,
    # before any DMA lands.
    dumw = const_pool.tile([128, 512], F16)
    nc.vector.memset(dumw[:], 0.0)
    # First 8 packs of P0 on DVE (blocks the very first build); the rest is
    # zeroed on GpSimd, interleaved with the builds below so the first build
    # leads the GpSimd queue.
    nc.vector.memset(P_tiles[0][:, :8 * 128], 0.0)

    in_pool = ctx.enter_context(tc.tile_pool(name="inp", bufs=1))
    x_pool = ctx.enter_context(tc.tile_pool(name="x32", bufs=4))
    mm_pool = ctx.enter_context(tc.tile_pool(name="mm", bufs=2, space="PSUM"))
    ob_pool = ctx.enter_context(tc.tile_pool(name="ob", bufs=8))

    # Per-supertile input tiles. Input DMA issues cost ~0.6us each on the
    # single Sync queue and their transfers contend with the output stream,
    # so only supertiles 0-1 load up front; supertile k+2 is loaded from
    # inside supertile k's loop, interleaved with the output DMAs (~30us of
    # lead time).
    in_tiles = []
    in_cols = []
    col = 0
    for si, npk in enumerate(NPKS):
        btT = in_pool.tile([128, npk * 36], F16, name=f"btT{si}")
        fxT = in_pool.tile([128, npk * 36], F16, name=f"fxT{si}")
        in_tiles.append((btT, fxT, 0))
        in_cols.append(col)
        col += npk * 36

    def load_inputs(si):
        btT, fxT, _ = in_tiles[si]
        cc = in_cols[si]
        n36 = NPKS[si] * 36
        nc.sync.dma_start(btT[:], btc[:, cc:cc + n36])
        nc.sync.dma_start(fxT[:], fxc[:, cc:cc + n36])

    load_inputs(0)
    load_inputs(1)

    # Warm up the PE (HAM un-throttle needs ~3.4us of busy) on the zeroed
    # dummy operand, starting right after the preamble.
    mmw = mm_pool.tile([128, 2048], FP, name="mmwarm", tag="mmgrp")
    for w in range(8):
        nc.tensor.matmul(mmw[:, (w % 4) * 512:(w % 4 + 1) * 512],
                         dumw[:, :128], dumw[:], start=True, stop=True)

    # Builds run on the DVE, split into chunks small enough to fit the
    # per-group slack between stream-transposes (ACT pace ~1.85us/group,
    # transposes ~0.6us/group).
    def build(si, lo, hi):
        btT, fxT, base = in_tiles[si]
        P4 = P_tiles[si % 2][:, lo * 128:hi * 128].rearrange(
            "p (pt g s) -> p pt g s", g=4, s=32)[:, :, :, 0:9]
        btv = btT[:, base + lo * 36:base + hi * 36].rearrange(
            "p (pt g j) -> p pt g j", g=4, j=9)
        fxv = fxT[:, base + lo * 36:base + hi * 36].rearrange(
            "p (pt g j) -> p pt g j", g=4, j=9)
        nc.vector.scalar_tensor_tensor(P4, btv, 1.0, fxv, op0=eq, op1=mult)

    build(0, 0, 8)
    nc.gpsimd.memset(P_tiles[0][:, 8 * 128:], 0.0)
    nc.gpsimd.memset(P_tiles[1][:], 0.0)
    # pending build chunks: (si, lo, hi), one emitted per compute group
    pend = [(0, lo, min(lo + 8, NPKS[0])) for lo in range(8, NPKS[0], 8)]
    ocol = 0
    for si, npk in enumerate(NPKS):
        P = P_tiles[si % 2]
        if si + 1 < len(NPKS):
            nxt = NPKS[si + 1]
            half = (nxt + 1) // 2
            pend += [(si + 1, 0, half), (si + 1, half, nxt)]
        gi = 0
        for blk in range(0, npk, 4):
            gn = min(4, npk - blk)
            dstep = 2 if (si == len(NPKS) - 1 and blk >= 16) else 4
            ob = ob_pool.tile([128, 2048], F16)
            mm = mm_pool.tile([128, 2048], FP, tag="mmgrp")
            x32 = x_pool.tile([128, 512], F16)
            nc.vector.transpose(x32[:, :gn * 128],
                                P[:, blk * 128:(blk + gn) * 128])
            for h in range(gn):
                nc.tensor.matmul(mm[:, h * 512:(h + 1) * 512],
                                 x32[:, h * 128:(h + 1) * 128],
                                 wbig_sb[:], start=True, stop=True)
            nc.scalar.activation(ob[:, :gn * 512], mm[:, :gn * 512],
                                 mybir.ActivationFunctionType.Tanh)
            if pend and (si == 0 or gi >= 1):
                build(*pend.pop(0))
            if blk == 4 and si + 2 < len(NPKS):
                load_inputs(si + 2)
            for d0 in range(0, gn, dstep):
                dn = min(dstep, gn - d0)
                oc = ocol + (blk + d0) * 512
                nc.sync.dma_start(out[:, oc:oc + dn * 512],
                                  ob[:, d0 * 512:(d0 + dn) * 512])
            gi += 1
        ocol += npk * 512


def build_nc():
    if "nc" in _NC_CACHE:
        return _NC_CACHE["nc"]
    nc = bacc.Bacc("TRN2", target_bir_lowering=False, debug=False)
    icols = sum(npk * 36 for npk in NPKS)
    ocols = sum(npk * 512 for npk in NPKS)
    btc = nc.dram_tensor("btc", [128, icols], F16, kind="ExternalInput").ap()
    fxc = nc.dram_tensor("fxc", [128, icols], F16, kind="ExternalInput").ap()
    wbig = nc.dram_tensor("wbig", [128, 512], F16, kind="ExternalInput").ap()
    out = nc.dram_tensor("out", [128, ocols], F16, kind="ExternalOutput").ap()
    with tile.TileContext(nc) as tc:
        with ExitStack() as ctx:
            _body(ctx, tc, out, btc, fxc, wbig)
    nc.compile()
    _NC_CACHE["nc"] = nc
    return nc


def make_wbig(W_slack, b_slack, W_gen, b_gen, W_load, b_load):
    W_list = [np.asarray(w, np.float32) for w in (W_slack, W_gen, W_load)]
    b_list = [np.asarray(b, np.float32) for b in (b_slack, b_gen, b_load)]
    WBig = np.zeros((128, 512), np.float16)
    for A in range(4):
        col = A * 128
        for e in range(3):
            WBig[32 * A + 3 * e + 0, col:col + 128] = \
                W_list[e][0].astype(np.float16)
            WBig[32 * A + 3 * e + 1, col:col + 128] = \
                W_list[e][1].astype(np.float16)
            WBig[32 * A + 3 * e + 2, col:col + 128] = \
                b_list[e].astype(np.float16)
    return WBig


def _permute_inputs(featp, btp):
    """featp (npad, 2) f32, btp (npad,) f32 -> btd, fxd (8, 128, 8856) f16
    in the device layout: per supertile, columns (pt, g, e, j) where
    btd = bus_type - e and fxd = [f0, f1, 1]."""
    featp = featp.reshape(N_CORES, PER_CORE, 2)
    btp = btp.reshape(N_CORES, PER_CORE)
    bparts, fparts = [], []
    off = 0
    erange = np.arange(3, dtype=np.float32)
    for ssz, npk in zip(SUPERS, NPKS):
        f4 = featp[:, off:off + ssz].reshape(N_CORES, 4, 32, npk, 4, 2)
        # orig (c, B, j, pt, A, k) -> device (c, p=32A+j, pt, g=B, k)
        dv = f4.transpose(0, 4, 2, 3, 1, 5).reshape(N_CORES, 128, npk, 4, 2)
        fx3 = np.empty((N_CORES, 128, npk, 4, 3), np.float16)
        fx3[..., :2] = dv
        fx3[..., 2] = 1.0
        fx9 = np.broadcast_to(fx3[:, :, :, :, None, :],
                              (N_CORES, 128, npk, 4, 3, 3))
        fparts.append(fx9.reshape(N_CORES, 128, npk * 36))
        b4 = btp[:, off:off + ssz].reshape(N_CORES, 4, 32, npk, 4)
        db = b4.transpose(0, 4, 2, 3, 1).reshape(N_CORES, 128, npk, 4)
        btE = (db[..., None] - erange)[..., None]
        btE = np.broadcast_to(btE, (N_CORES, 128, npk, 4, 3, 3))
        bparts.append(btE.astype(np.float16).reshape(N_CORES, 128, npk * 36))
        off += ssz
    btd = np.ascontiguousarray(np.concatenate(bparts, axis=2))
    fxd = np.ascontiguousarray(np.concatenate(fparts, axis=2))
    return btd, fxd


def kernel(feat, bus_type, W_slack, b_slack, W_gen, b_gen, W_load, b_load,
           **run_kwargs):
    feat = np.asarray(feat, np.float32)
    bt = np.asarray(bus_type)
    n = feat.shape[0]
    npad = N_CORES * PER_CORE
    assert n <= npad

    featp = np.zeros((npad, 2), np.float32)
    featp[:n] = feat
    btp = np.zeros(npad, np.float32)
    btp[:n] = bt.astype(np.float32)
    btd, fxd = _permute_inputs(featp, btp)
    wbig = make_wbig(W_slack, b_slack, W_gen, b_gen, W_load, b_load)

    nc = build_nc()
    in_maps = [
        {"btc": btd[i], "fxc": fxd[i], "wbig": wbig}
        for i in range(N_CORES)
    ]
    try:
        res = run_bass_kernel_spmd(nc, in_maps, list(range(N_CORES)),
                                   **run_kwargs)
    except Exception:
        # A previously-failed process can leave the NeuronCores wedged
        # (NRT_EXEC_UNIT_UNRECOVERABLE); a small probe op resets them.
        import time as _time

        import jax.numpy as jnp

        for _ in range(3):
            try:
                float(jnp.sum(jnp.ones((8, 8))))
                break
            except Exception:
                _time.sleep(5)
        res = run_bass_kernel_spmd(nc, in_maps, list(range(N_CORES)),
                                   **run_kwargs)

    outs = []
    for i in range(N_CORES):
        dev = res.results[i]["out"]  # (128, 125952) f16
        off = 0
        parts = []
        for ssz, npk in zip(SUPERS, NPKS):
            block = dev[:, off:off + npk * 512].reshape(128, npk, 4, 128)
            parts.append(block.reshape(ssz, D))
            off += npk * 512
        outs.append(np.concatenate(parts, axis=0))
    out = np.concatenate(outs, axis=0)
    kernel.last_result = res
    return out[:n].astype(np.float32)


# revision 29
# speedup vs baseline: 1.0039x; 1.0039x over previous
"""V4: fp16-output MoE routing kernel.

The correctness gate is rel_err < 2e-2; plain fp16 arithmetic gives ~1.4e-3,
so no Dekker splits; fp16 output halves HBM write traffic. Engine budget per
core (125,952 tokens): ACT tanh ~116us gapless phase (bottleneck), DMA
~107us, DVE ~60us, PE ~55us. v4 focuses on the startup ramp (22.9us in v3)
and tail.

Data flow per 512-token pack:
- P[p, 32g+s] fp16 slot matrix, 9 slots used per 32-slot group, built by a
  SINGLE scalar_tensor_tensor per supertile: (btE == 1) * fx, where the host
  ships btE = bus_type - e and fx = [f0, f1, 1] triples per expert block so
  the DVE write runs are 9 contiguous elements (s = 3e+j).
- DVE stream-transpose (32x32 blocks): X32[32A+i, 32B+j] = P[32A+j, 32B+i]
- matmul(out, lhsT=X32, rhs=Wbig) with Wbig block-diagonal over A-blocks:
    out[32B+j, A*128+d] = z(token(p=32A+j, g=B), d)    (one PSUM bank/pack)
- ACT tanh over 4 packs (FD=2048, 4 PSUM banks double-buffered), fp16 SBUF
- output dumped linearly [128, npk*512]; the host input permutation is
  chosen so per-partition HBM lines are contiguous and the host-side unpack
  is a pure reshape: orig token q*(npk*4) + pt*4 + A sits at device slot
  (p = 32A + (q%32), pack pt, g = q//32).
"""

import sys
from contextlib import ExitStack

import numpy as np

sys.path.insert(0, "/opt/trn_rl_repo")

import concourse.bacc as bacc  # noqa: E402
import concourse.mybir as mybir  # noqa: E402
import concourse.tile as tile  # noqa: E402
from concourse.bass_utils import run_bass_kernel_spmd  # noqa: E402

FP = mybir.dt.float32
F16 = mybir.dt.float16
D = 128
PACK = 512
SUPERS = [16384] * 7 + [11264]
NPKS = [s // PACK for s in SUPERS]
N_CORES = 8
PER_CORE = sum(SUPERS)

_NC_CACHE = {}


def _body(ctx, tc, out, btc, fxc, wbig):
    nc = tc.nc
    eq = mybir.AluOpType.is_equal
    mult = mybir.AluOpType.mult

    const_pool = ctx.enter_context(tc.tile_pool(name="const", bufs=1))
    wbig_sb = const_pool.tile([128, 512], F16)
    nc.sync.dma_start(wbig_sb[:], wbig)
    # Persistent double-buffered slot matrices; slots 9..31 are zeroed once
    # and never written again (their Wbig rows are zero, but NaN garbage
    # would still poison the accumulation, so the memset is required).
    P_tiles = [const_pool.tile([128, 32 * 128], F16, name=f"Pbuf{i}")
               for i in range(2)]
    # Zeroed dummy operand lets PE warmup start right after the preamble,
    # before any DMA lands.
    dumw = const_pool.tile([128, 512], F16)
    nc.vector.memset(dumw[:], 0.0)
    # First 8 packs of P0 on DVE (blocks the very first build); the rest is
    # zeroed on GpSimd, interleaved with the builds below so the first build
    # leads the GpSimd queue.
    nc.vector.memset(P_tiles[0][:, :8 * 128], 0.0)

    in_pool = ctx.enter_context(tc.tile_pool(name="inp", bufs=1))
    x_pool = ctx.enter_context(tc.tile_pool(name="x32", bufs=4))
    mm_pool = ctx.enter_context(tc.tile_pool(name="mm", bufs=2, space="PSUM"))
    ob_pool = ctx.enter_context(tc.tile_pool(name="ob", bufs=8))

    # Per-supertile input tiles. Input DMA issues cost ~0.6us each on the
    # single Sync queue and their transfers contend with the output stream,
    # so only supertiles 0-1 load up front; supertile k+2 is loaded from
    # inside supertile k's loop, interleaved with the output DMAs (~30us of
    # lead time).
    in_tiles = []
    in_cols = []
    col = 0
    for si, npk in enumerate(NPKS):
        btT = in_pool.tile([128, npk * 36], F16, name=f"btT{si}")
        fxT = in_pool.tile([128, npk * 36], F16, name=f"fxT{si}")
        in_tiles.append((btT, fxT, 0))
        in_cols.append(col)
        col += npk * 36

    def load_inputs(si):
        btT, fxT, _ = in_tiles[si]
        cc = in_cols[si]
        n36 = NPKS[si] * 36
        nc.sync.dma_start(btT[:], btc[:, cc:cc + n36])
        nc.sync.dma_start(fxT[:], fxc[:, cc:cc + n36])

    load_inputs(0)
    load_inputs(1)

    # Warm up the PE (HAM un-throttle needs ~3.4us of busy) on the zeroed
    # dummy operand, starting right after the preamble.
    mmw = mm_pool.tile([128, 2048], FP, name="mmwarm", tag="mmgrp")
    for w in range(8):
        nc.tensor.matmul(mmw[:, (w % 4) * 512:(w % 4 + 1) * 512],
                         dumw[:, :128], dumw[:], start=True, stop=True)

    # Builds run on the DVE, split into chunks small enough to fit the
    # per-group slack between stream-transposes (ACT pace ~1.85us/group,
    # transposes ~0.6us/group).
    def build(si, lo, hi):
        btT, fxT, base = in_tiles[si]
        P4 = P_tiles[si % 2][:, lo * 128:hi * 128].rearrange(
            "p (pt g s) -> p pt g s", g=4, s=32)[:, :, :, 0:9]
        btv = btT[:, base + lo * 36:base + hi * 36].rearrange(
            "p (pt g j) -> p pt g j", g=4, j=9)
        fxv = fxT[:, base + lo * 36:base + hi * 36].rearrange(
            "p (pt g j) -> p pt g j", g=4, j=9)
        nc.vector.scalar_tensor_tensor(P4, btv, 1.0, fxv, op0=eq, op1=mult)

    build(0, 0, 8)
    nc.gpsimd.memset(P_tiles[0][:, 8 * 128:], 0.0)
    nc.gpsimd.memset(P_tiles[1][:], 0.0)
    # pending build chunks: (si, lo, hi), one emitted per compute group.
    # The next supertile's chunks are held back until late in the current
    # supertile (gi>=5): the scheduler coalesces upcoming DVE waits into one
    # EVENT_SEMAPHORE, and emitting them early would make the first builds
    # wait on the next supertile's input DMAs.
    pend = [(0, lo, min(lo + 8, NPKS[0])) for lo in range(8, NPKS[0], 8)]
    nxt_pend = []
    ocol = 0
    for si, npk in enumerate(NPKS):
        P = P_tiles[si % 2]
        pend += nxt_pend
        nxt_pend = []
        if si + 1 < len(NPKS):
            nxt = NPKS[si + 1]
            half = (nxt + 1) // 2
            nxt_pend = [(si + 1, 0, half), (si + 1, half, nxt)]
        gi = 0
        for blk in range(0, npk, 4):
            gn = min(4, npk - blk)
            dstep = 2 if (si == len(NPKS) - 1 and blk >= 16) else 4
            ob = ob_pool.tile([128, 2048], F16)
            mm = mm_pool.tile([128, 2048], FP, tag="mmgrp")
            x32 = x_pool.tile([128, 512], F16)
            nc.vector.transpose(x32[:, :gn * 128],
                                P[:, blk * 128:(blk + gn) * 128])
            for h in range(gn):
                nc.tensor.matmul(mm[:, h * 512:(h + 1) * 512],
                                 x32[:, h * 128:(h + 1) * 128],
                                 wbig_sb[:], start=True, stop=True)
            nc.scalar.activation(ob[:, :gn * 512], mm[:, :gn * 512],
                                 mybir.ActivationFunctionType.Tanh)
            if pend:
                build(*pend.pop(0))
            elif nxt_pend and gi >= 5:
                build(*nxt_pend.pop(0))
            if blk == 4 and si + 2 < len(NPKS):
                load_inputs(si + 2)
            for d0 in range(0, gn, dstep):
                dn = min(dstep, gn - d0)
                oc = ocol + (blk + d0) * 512
                nc.sync.dma_start(out[:, oc:oc + dn * 512],
                                  ob[:, d0 * 512:(d0 + dn) * 512])
            gi += 1
        ocol += npk * 512


def build_nc():
    if "nc" in _NC_CACHE:
        return _NC_CACHE["nc"]
    nc = bacc.Bacc("TRN2", target_bir_lowering=False, debug=False)
    icols = sum(npk * 36 for npk in NPKS)
    ocols = sum(npk * 512 for npk in NPKS)
    btc = nc.dram_tensor("btc", [128, icols], F16, kind="ExternalInput").ap()
    fxc = nc.dram_tensor("fxc", [128, icols], F16, kind="ExternalInput").ap()
    wbig = nc.dram_tensor("wbig", [128, 512], F16, kind="ExternalInput").ap()
    out = nc.dram_tensor("out", [128, ocols], F16, kind="ExternalOutput").ap()
    with tile.TileContext(nc) as tc:
        with ExitStack() as ctx:
            _body(ctx, tc, out, btc, fxc, wbig)
    nc.compile()
    _NC_CACHE["nc"] = nc
    return nc


def make_wbig(W_slack, b_slack, W_gen, b_gen, W_load, b_load):
    W_list = [np.asarray(w, np.float32) for w in (W_slack, W_gen, W_load)]
    b_list = [np.asarray(b, np.float32) for b in (b_slack, b_gen, b_load)]
    WBig = np.zeros((128, 512), np.float16)
    for A in range(4):
        col = A * 128
        for e in range(3):
            WBig[32 * A + 3 * e + 0, col:col + 128] = \
                W_list[e][0].astype(np.float16)
            WBig[32 * A + 3 * e + 1, col:col + 128] = \
                W_list[e][1].astype(np.float16)
            WBig[32 * A + 3 * e + 2, col:col + 128] = \
                b_list[e].astype(np.float16)
    return WBig


def _permute_inputs(featp, btp):
    """featp (npad, 2) f32, btp (npad,) f32 -> btd, fxd (8, 128, 8856) f16
    in the device layout: per supertile, columns (pt, g, e, j) where
    btd = bus_type - e and fxd = [f0, f1, 1]."""
    featp = featp.reshape(N_CORES, PER_CORE, 2)
    btp = btp.reshape(N_CORES, PER_CORE)
    bparts, fparts = [], []
    off = 0
    erange = np.arange(3, dtype=np.float32)
    for ssz, npk in zip(SUPERS, NPKS):
        f4 = featp[:, off:off + ssz].reshape(N_CORES, 4, 32, npk, 4, 2)
        # orig (c, B, j, pt, A, k) -> device (c, p=32A+j, pt, g=B, k)
        dv = f4.transpose(0, 4, 2, 3, 1, 5).reshape(N_CORES, 128, npk, 4, 2)
        fx3 = np.empty((N_CORES, 128, npk, 4, 3), np.float16)
        fx3[..., :2] = dv
        fx3[..., 2] = 1.0
        fx9 = np.broadcast_to(fx3[:, :, :, :, None, :],
                              (N_CORES, 128, npk, 4, 3, 3))
        fparts.append(fx9.reshape(N_CORES, 128, npk * 36))
        b4 = btp[:, off:off + ssz].reshape(N_CORES, 4, 32, npk, 4)
        db = b4.transpose(0, 4, 2, 3, 1).reshape(N_CORES, 128, npk, 4)
        btE = (db[..., None] - erange)[..., None]
        btE = np.broadcast_to(btE, (N_CORES, 128, npk, 4, 3, 3))
        bparts.append(btE.astype(np.float16).reshape(N_CORES, 128, npk * 36))
        off += ssz
    btd = np.ascontiguousarray(np.concatenate(bparts, axis=2))
    fxd = np.ascontiguousarray(np.concatenate(fparts, axis=2))
    return btd, fxd


def kernel(feat, bus_type, W_slack, b_slack, W_gen, b_gen, W_load, b_load,
           **run_kwargs):
    feat = np.asarray(feat, np.float32)
    bt = np.asarray(bus_type)
    n = feat.shape[0]
    npad = N_CORES * PER_CORE
    assert n <= npad

    featp = np.zeros((npad, 2), np.float32)
    featp[:n] = feat
    btp = np.zeros(npad, np.float32)
    btp[:n] = bt.astype(np.float32)
    btd, fxd = _permute_inputs(featp, btp)
    wbig = make_wbig(W_slack, b_slack, W_gen, b_gen, W_load, b_load)

    nc = build_nc()
    in_maps = [
        {"btc": btd[i], "fxc": fxd[i], "wbig": wbig}
        for i in range(N_CORES)
    ]
    try:
        res = run_bass_kernel_spmd(nc, in_maps, list(range(N_CORES)),
                                   **run_kwargs)
    except Exception:
        # A previously-failed process can leave the NeuronCores wedged
        # (NRT_EXEC_UNIT_UNRECOVERABLE); a small probe op resets them.
        import time as _time

        import jax.numpy as jnp

        for _ in range(3):
            try:
                float(jnp.sum(jnp.ones((8, 8))))
                break
            except Exception:
                _time.sleep(5)
        res = run_bass_kernel_spmd(nc, in_maps, list(range(N_CORES)),
                                   **run_kwargs)

    outs = []
    for i in range(N_CORES):
        dev = res.results[i]["out"]  # (128, 125952) f16
        off = 0
        parts = []
        for ssz, npk in zip(SUPERS, NPKS):
            block = dev[:, off:off + npk * 512].reshape(128, npk, 4, 128)
            parts.append(block.reshape(ssz, D))
            off += npk * 512
        outs.append(np.concatenate(parts, axis=0))
    out = np.concatenate(outs, axis=0)
    kernel.last_result = res
    return out[:n].astype(np.float32)


# revision 33
# speedup vs baseline: 1.0116x; 1.0077x over previous
"""V4: fp16-output MoE routing kernel.

The correctness gate is rel_err < 2e-2; plain fp16 arithmetic gives ~1.4e-3,
so no Dekker splits; fp16 output halves HBM write traffic. Engine budget per
core (125,952 tokens): ACT tanh ~116us gapless phase (bottleneck), DMA
~107us, DVE ~60us, PE ~55us. v4 focuses on the startup ramp (22.9us in v3)
and tail.

Data flow per 512-token pack:
- P[p, 32g+s] fp16 slot matrix, 9 slots used per 32-slot group, built by a
  SINGLE scalar_tensor_tensor per supertile: (btE == 1) * fx, where the host
  ships btE = bus_type - e and fx = [f0, f1, 1] triples per expert block so
  the DVE write runs are 9 contiguous elements (s = 3e+j).
- DVE stream-transpose (32x32 blocks): X32[32A+i, 32B+j] = P[32A+j, 32B+i]
- matmul(out, lhsT=X32, rhs=Wbig) with Wbig block-diagonal over A-blocks:
    out[32B+j, A*128+d] = z(token(p=32A+j, g=B), d)    (one PSUM bank/pack)
- ACT tanh over 4 packs (FD=2048, 4 PSUM banks double-buffered), fp16 SBUF
- output dumped linearly [128, npk*512]; the host input permutation is
  chosen so per-partition HBM lines are contiguous and the host-side unpack
  is a pure reshape: orig token q*(npk*4) + pt*4 + A sits at device slot
  (p = 32A + (q%32), pack pt, g = q//32).
"""

import sys
from contextlib import ExitStack

import numpy as np

sys.path.insert(0, "/opt/trn_rl_repo")

import concourse.bacc as bacc  # noqa: E402
import concourse.mybir as mybir  # noqa: E402
import concourse.tile as tile  # noqa: E402
from concourse.bass_utils import run_bass_kernel_spmd  # noqa: E402

FP = mybir.dt.float32
F16 = mybir.dt.float16
D = 128
PACK = 512
SUPERS = [16384] * 7 + [11264]
NPKS = [s // PACK for s in SUPERS]
N_CORES = 8
PER_CORE = sum(SUPERS)

_NC_CACHE = {}


def _body(ctx, tc, out, btc, fxc, wbig):
    nc = tc.nc
    eq = mybir.AluOpType.is_equal
    mult = mybir.AluOpType.mult

    const_pool = ctx.enter_context(tc.tile_pool(name="const", bufs=1))
    wbig_sb = const_pool.tile([128, 512], F16)
    nc.sync.dma_start(wbig_sb[:], wbig)
    # Persistent double-buffered slot matrices; slots 9..31 are zeroed once
    # and never written again (their Wbig rows are zero, but NaN garbage
    # would still poison the accumulation, so the memset is required).
    P_tiles = [const_pool.tile([128, 32 * 128], F16, name=f"Pbuf{i}")
               for i in range(2)]
    # Zeroed dummy operand lets PE warmup start right after the preamble,
    # before any DMA lands.
    dumw = const_pool.tile([128, 512], F16)
    nc.vector.memset(dumw[:], 0.0)
    # First 8 packs of P0 on DVE (blocks the very first build); the rest is
    # zeroed on GpSimd, interleaved with the builds below so the first build
    # leads the GpSimd queue.
    nc.vector.memset(P_tiles[0][:, :8 * 128], 0.0)

    in_pool = ctx.enter_context(tc.tile_pool(name="inp", bufs=1))
    x_pool = ctx.enter_context(tc.tile_pool(name="x32", bufs=4))
    mm_pool = ctx.enter_context(tc.tile_pool(name="mm", bufs=2, space="PSUM"))
    ob_pool = ctx.enter_context(tc.tile_pool(name="ob", bufs=8))

    # Per-supertile input tiles. Input DMA issues cost ~0.6us each on the
    # single Sync queue and their transfers contend with the output stream,
    # so only supertiles 0-1 load up front; supertile k+2 is loaded from
    # inside supertile k's loop, interleaved with the output DMAs (~30us of
    # lead time).
    in_tiles = []
    bcol = fcol = 0
    for si, npk in enumerate(NPKS):
        btT = in_pool.tile([128, npk * 4], F16, name=f"btT{si}")
        fxT = in_pool.tile([128, npk * 12], F16, name=f"fxT{si}")
        in_tiles.append((btT, fxT, bcol, fcol))
        bcol += npk * 4
        fcol += npk * 12

    def load_inputs(si):
        btT, fxT, bc, fc = in_tiles[si]
        npk = NPKS[si]
        nc.sync.dma_start(btT[:], btc[:, bc:bc + npk * 4])
        nc.sync.dma_start(fxT[:], fxc[:, fc:fc + npk * 12])

    load_inputs(0)
    load_inputs(1)

    # Warm up the PE (HAM un-throttle needs ~3.4us of busy) on the zeroed
    # dummy operand, starting right after the preamble.
    mmw = mm_pool.tile([128, 2048], FP, name="mmwarm", tag="mmgrp")
    for w in range(8):
        nc.tensor.matmul(mmw[:, (w % 4) * 512:(w % 4 + 1) * 512],
                         dumw[:, :128], dumw[:], start=True, stop=True)

    # Builds run on the DVE, split into chunks small enough to fit the
    # per-group slack between stream-transposes (ACT pace ~1.85us/group,
    # transposes ~0.6us/group).
    def build(si, lo, hi):
        btT, fxT, _, _ = in_tiles[si]
        pk = hi - lo
        P4 = P_tiles[si % 2][:, lo * 128:hi * 128].rearrange(
            "p (pt g s) -> p pt g s", g=4, s=32)
        btv = btT[:, lo * 4:hi * 4].rearrange(
            "p (pt g) -> p pt g", g=4).broadcast_to([128, pk, 4, 3])
        fxv = fxT[:, lo * 12:hi * 12].rearrange(
            "p (pt g j) -> p pt g j", g=4, j=3)
        for e in range(3):
            nc.vector.scalar_tensor_tensor(
                P4[:, :, :, 3 * e:3 * e + 3], btv, float(e + 1), fxv,
                op0=eq, op1=mult)

    build(0, 0, 8)
    nc.gpsimd.memset(P_tiles[0][:, 8 * 128:], 0.0)
    nc.gpsimd.memset(P_tiles[1][:], 0.0)
    # pending build chunks: (si, lo, hi), one emitted per compute group.
    # The next supertile's chunks are held back until late in the current
    # supertile (gi>=5): the scheduler coalesces upcoming DVE waits into one
    # EVENT_SEMAPHORE, and emitting them early would make the first builds
    # wait on the next supertile's input DMAs.
    pend = [(0, lo, min(lo + 8, NPKS[0])) for lo in range(8, NPKS[0], 8)]
    nxt_pend = []
    ocol = 0
    for si, npk in enumerate(NPKS):
        P = P_tiles[si % 2]
        pend += nxt_pend
        nxt_pend = []
        if si + 1 < len(NPKS):
            nxt = NPKS[si + 1]
            half = (nxt + 1) // 2
            nxt_pend = [(si + 1, 0, half), (si + 1, half, nxt)]
        gi = 0
        for blk in range(0, npk, 4):
            gn = min(4, npk - blk)
            dstep = 2 if (si == len(NPKS) - 1 and blk >= 16) else 4
            ob = ob_pool.tile([128, 2048], F16)
            mm = mm_pool.tile([128, 2048], FP, tag="mmgrp")
            x32 = x_pool.tile([128, 512], F16)
            nc.vector.transpose(x32[:, :gn * 128],
                                P[:, blk * 128:(blk + gn) * 128])
            for h in range(gn):
                nc.tensor.matmul(mm[:, h * 512:(h + 1) * 512],
                                 x32[:, h * 128:(h + 1) * 128],
                                 wbig_sb[:], start=True, stop=True)
            nc.scalar.activation(ob[:, :gn * 512], mm[:, :gn * 512],
                                 mybir.ActivationFunctionType.Tanh)
            if pend:
                build(*pend.pop(0))
            elif nxt_pend and gi >= 5:
                build(*nxt_pend.pop(0))
            if blk == 4 and si + 2 < len(NPKS):
                load_inputs(si + 2)
            for d0 in range(0, gn, dstep):
                dn = min(dstep, gn - d0)
                oc = ocol + (blk + d0) * 512
                nc.sync.dma_start(out[:, oc:oc + dn * 512],
                                  ob[:, d0 * 512:(d0 + dn) * 512])
            gi += 1
        ocol += npk * 512


def build_nc():
    if "nc" in _NC_CACHE:
        return _NC_CACHE["nc"]
    nc = bacc.Bacc("TRN2", target_bir_lowering=False, debug=False)
    bcols = sum(npk * 4 for npk in NPKS)
    fcols = sum(npk * 12 for npk in NPKS)
    ocols = sum(npk * 512 for npk in NPKS)
    btc = nc.dram_tensor("btc", [128, bcols], F16, kind="ExternalInput").ap()
    fxc = nc.dram_tensor("fxc", [128, fcols], F16, kind="ExternalInput").ap()
    wbig = nc.dram_tensor("wbig", [128, 512], F16, kind="ExternalInput").ap()
    out = nc.dram_tensor("out", [128, ocols], F16, kind="ExternalOutput").ap()
    with tile.TileContext(nc) as tc:
        with ExitStack() as ctx:
            _body(ctx, tc, out, btc, fxc, wbig)
    nc.compile()
    _NC_CACHE["nc"] = nc
    return nc


def make_wbig(W_slack, b_slack, W_gen, b_gen, W_load, b_load):
    W_list = [np.asarray(w, np.float32) for w in (W_slack, W_gen, W_load)]
    b_list = [np.asarray(b, np.float32) for b in (b_slack, b_gen, b_load)]
    WBig = np.zeros((128, 512), np.float16)
    for A in range(4):
        col = A * 128
        for e in range(3):
            WBig[32 * A + 3 * e + 0, col:col + 128] = \
                W_list[e][0].astype(np.float16)
            WBig[32 * A + 3 * e + 1, col:col + 128] = \
                W_list[e][1].astype(np.float16)
            WBig[32 * A + 3 * e + 2, col:col + 128] = \
                b_list[e].astype(np.float16)
    return WBig


def _permute_inputs(featp, btp):
    """featp (npad, 2) f32, btp (npad,) f32 -> btd (8, 128, 984) f16 holding
    raw bus_type and fxd (8, 128, 2952) f16 holding [f0, f1, 1] triples, in
    the device layout (p, pt, g)."""
    featp = featp.reshape(N_CORES, PER_CORE, 2)
    btp = btp.reshape(N_CORES, PER_CORE)
    bparts, fparts = [], []
    off = 0
    for ssz, npk in zip(SUPERS, NPKS):
        f4 = featp[:, off:off + ssz].reshape(N_CORES, 4, 32, npk, 4, 2)
        # orig (c, B, j, pt, A, k) -> device (c, p=32A+j, pt, g=B, k)
        dv = f4.transpose(0, 4, 2, 3, 1, 5).reshape(N_CORES, 128, npk, 4, 2)
        fx3 = np.empty((N_CORES, 128, npk, 4, 3), np.float16)
        fx3[..., :2] = dv
        fx3[..., 2] = 1.0
        fparts.append(fx3.reshape(N_CORES, 128, npk * 12))
        b4 = btp[:, off:off + ssz].reshape(N_CORES, 4, 32, npk, 4)
        db = b4.transpose(0, 4, 2, 3, 1).reshape(N_CORES, 128, npk, 4)
        bparts.append(db.astype(np.float16).reshape(N_CORES, 128, npk * 4))
        off += ssz
    btd = np.ascontiguousarray(np.concatenate(bparts, axis=2))
    fxd = np.ascontiguousarray(np.concatenate(fparts, axis=2))
    return btd, fxd


def kernel(feat, bus_type, W_slack, b_slack, W_gen, b_gen, W_load, b_load,
           **run_kwargs):
    feat = np.asarray(feat, np.float32)
    bt = np.asarray(bus_type)
    n = feat.shape[0]
    npad = N_CORES * PER_CORE
    assert n <= npad

    featp = np.zeros((npad, 2), np.float32)
    featp[:n] = feat
    btp = np.zeros(npad, np.float32)
    btp[:n] = bt.astype(np.float32)
    btd, fxd = _permute_inputs(featp, btp)
    wbig = make_wbig(W_slack, b_slack, W_gen, b_gen, W_load, b_load)

    nc = build_nc()
    in_maps = [
        {"btc": btd[i], "fxc": fxd[i], "wbig": wbig}
        for i in range(N_CORES)
    ]
    try:
        res = run_bass_kernel_spmd(nc, in_maps, list(range(N_CORES)),
                                   **run_kwargs)
    except Exception:
        # A previously-failed process can leave the NeuronCores wedged
        # (NRT_EXEC_UNIT_UNRECOVERABLE); a small probe op resets them.
        import time as _time

        import jax.numpy as jnp

        for _ in range(3):
            try:
                float(jnp.sum(jnp.ones((8, 8))))
                break
            except Exception:
                _time.sleep(5)
        res = run_bass_kernel_spmd(nc, in_maps, list(range(N_CORES)),
                                   **run_kwargs)

    outs = []
    for i in range(N_CORES):
        dev = res.results[i]["out"]  # (128, 125952) f16
        off = 0
        parts = []
        for ssz, npk in zip(SUPERS, NPKS):
            block = dev[:, off:off + npk * 512].reshape(128, npk, 4, 128)
            parts.append(block.reshape(ssz, D))
            off += npk * 512
        outs.append(np.concatenate(parts, axis=0))
    out = np.concatenate(outs, axis=0)
    kernel.last_result = res
    return out[:n].astype(np.float32)
